# revision 1
# baseline (speedup 1.0000x reference)
"""Trainium2 Bass kernel for nn_MultiHeadAttention3_549755814010.

Math note: softmax over a length-1 key axis is identically 1.0, so the
reference reduces to

    S_b     = sum_d v[b, d]                                  (per-batch scalar)
    z[b,:]  = S_b * v[b,:] + k[b,:]                          (2048, 640)
    y[b,:]  = z[b,:] @ w_fc.T + b_fc                         (small matmul)
    wg[b,:] = y[b,:] * gamma1                                (2048, 640)
    out[b,q,:] = LayerNorm(wg[b,:] + q[b,q,:]) * ln_w + ln_b (the bulk)

Strategy (~113-135us/core vs ~143us f32 baseline; HW run-to-run
variance on this part is ~+/-10%):
  * bf16 I/O (q cast host-side, out stored bf16, upcast host-side):
    halves both HBM directions -> DMA floor ~65us/core incl. consts.
    LN stats stay f32; measured rel_linf ~6.6e-3 vs the 2e-2 gate.
  * Per-segment work is split across three engine "routes" (see the
    R4_GROUPS/R3_GROUPS knobs below) so no single engine saturates:
    r1, r3, r4 as documented at the knobs.
  * All 8 q tiles have dedicated SBUF slots: loads stream back-to-back
    at full rate (~354 GB/s measured on the sync HWDGE ring).

Known environment hazards: raw bass.Bass lacks the multi-wait splitting
passes (use Bacc); tensor_tensor_reduce and qpool bufs=7 crash the
device; scalar_tensor_tensor is invalid on GPSIMD; matmul PSUM dest
must fit one 2KB bank (<=512 f32); a group must not hold more live
psum_x tiles than the pool has bufs (deadlock).
"""

import numpy as np
from contextlib import ExitStack

import ml_dtypes

import concourse.bass as bass
import concourse.tile as tile
from concourse import bacc, mybir
from concourse.bass_utils import run_bass_kernel_spmd

N_CORES = 8
NUM_C, LQ, D = 2048, 32, 640
B = NUM_C // N_CORES          # 256 batches per core
H = B // 128                  # 2 batch halves of 128 (partition dim)
SEG = 8                       # qpos positions per tile
NJ = LQ // SEG                # 4 qpos chunks per batch half
EPS_LN = 1e-5
F32 = mybir.dt.float32
BF16 = mybir.dt.bfloat16
AX = mybir.AxisListType
ALU = mybir.AluOpType
ACTF = mybir.ActivationFunctionType

# Route knobs, per (tile_idx, group_idx):
#  r1: DVE STT add (+s1 accum), ACT Square (+s2 accum), GPSIMD normalize
#  r3: DVE add, DVE bn_stats pair on SBUF x, GPSIMD normalize
#  r4: PE identity-matmul add into PSUM, DVE bn_stats pair on PSUM,
#      ACT normalize from PSUM (clustered on tiles 1/3/5 to keep PE warm)
R4_GROUPS = {(t, g) for t in (1, 3, 5) for g in range(4)}
R3_GROUPS = {(0, 2), (2, 2), (4, 2), (6, 2), (0, 3), (4, 3)}


def _build(ln_trivial: bool) -> bass.Bass:
    nc = bacc.Bacc("TRN2", name="mha3_549755814010")

    q = nc.dram_tensor("q", (B, LQ * D), BF16, kind="ExternalInput")
    vv = nc.dram_tensor("vv", (B, D), BF16, kind="ExternalInput")
    vT = nc.dram_tensor("vT", (128, 5, B), BF16, kind="ExternalInput")
    kT = nc.dram_tensor("kT", (128, 5, B), BF16, kind="ExternalInput")
    wgw = nc.dram_tensor("wgw", (128, 5, D), BF16, kind="ExternalInput")
    wgb = nc.dram_tensor("wgb", (1, D), BF16, kind="ExternalInput")
    ident = nc.dram_tensor("ident", (128, 128), BF16, kind="ExternalInput")
    if not ln_trivial:
        lnw = nc.dram_tensor("lnw", (1, D), F32, kind="ExternalInput")
        lnb = nc.dram_tensor("lnb", (1, D), F32, kind="ExternalInput")
    o = nc.dram_tensor("o", (B, LQ * D), BF16, kind="ExternalOutput")

    with ExitStack() as ctx:
        tc = ctx.enter_context(tile.TileContext(nc))
        const = ctx.enter_context(tc.tile_pool(name="const", bufs=1))
        work = ctx.enter_context(tc.tile_pool(name="work", bufs=4))
        qpool = ctx.enter_context(tc.tile_pool(name="qpool", bufs=8))
        stat = ctx.enter_context(tc.tile_pool(name="stat", bufs=4))
        psum_y = ctx.enter_context(tc.tile_pool(name="psum_y", bufs=1, space="PSUM"))
        psum_x = ctx.enter_context(tc.tile_pool(name="psum_x", bufs=3, space="PSUM"))

        # ---- constants ----
        ones_row = const.tile([1, 128], BF16)
        nc.vector.memset(ones_row, 1.0)
        eps_t = const.tile([128, 1], F32)
        nc.vector.memset(eps_t, EPS_LN)

        wgw_sb = const.tile([128, 5, D], BF16)
        vTt = const.tile([128, 5, B], BF16)
        kTt = const.tile([128, 5, B], BF16)
        wgb_sb = const.tile([1, D], BF16)
        vt = const.tile([128, H, D], BF16)
        id_sb = const.tile([128, 128], BF16)
        if not ln_trivial:
            lnw_b = const.tile([128, D], BF16)
            lnb_b = const.tile([128, D], BF16)
        with tc.high_priority():
            nc.sync.dma_start(out=vTt, in_=vT[:, :, :])
            nc.sync.dma_start(out=kTt, in_=kT[:, :, :])
            nc.sync.dma_start(out=wgw_sb, in_=wgw[:, :, :])
            nc.sync.dma_start(out=id_sb, in_=ident[:, :])
            for h in range(H):
                nc.sync.dma_start(out=vt[:, h, :],
                                  in_=vv[h * 128:(h + 1) * 128, :])
            nc.sync.dma_start(out=wgb_sb, in_=wgb[:, :])
            if not ln_trivial:
                nc.sync.dma_start(out=lnw_b, in_=lnw.to_broadcast((128, D)))
                nc.sync.dma_start(out=lnb_b, in_=lnb.to_broadcast((128, D)))

        # ---- PE warmup: dummy matmuls so the wg matmuls run at full clock
        warm = const.tile([128, 512], BF16)
        nc.vector.memset(warm, 1.0)
        with tc.high_priority():
            for _ in range(12):
                pw = psum_y.tile([128, 320], F32, tag="pv")
                nc.tensor.matmul(pw, lhsT=warm[:, 0:128], rhs=warm[:, 0:320],
                                 start=True, stop=True)

        # ---- prologue: wg = (S*v + k) @ W + b_fc*gamma  (W = w_fc.T*gamma1)
        sv = const.tile([128, H], F32)
        for h in range(H):
            nc.vector.reduce_sum(out=sv[:, h:h + 1], in_=vt[:, h, :],
                                 axis=AX.X)

        wg = const.tile([128, H, D], BF16)
        for h in range(H):
            hsl = slice(h * 128, (h + 1) * 128)
            for oo in range(2):
                osl = slice(oo * 320, (oo + 1) * 320)
                pv = psum_y.tile([128, 320], F32, tag="pv")
                for c in range(5):
                    nc.tensor.matmul(pv, lhsT=vTt[:, c, hsl],
                                     rhs=wgw_sb[:, c, osl],
                                     start=(c == 0), stop=(c == 4))
                pk = psum_y.tile([128, 320], F32, tag="pk")
                for c in range(5):
                    nc.tensor.matmul(pk, lhsT=kTt[:, c, hsl],
                                     rhs=wgw_sb[:, c, osl],
                                     start=(c == 0), stop=False)
                nc.tensor.matmul(pk, lhsT=ones_row[:, :],
                                 rhs=wgb_sb[:, osl], start=False, stop=True)
                nc.vector.tensor_scalar(out=wg[:, h, osl], in0=pv,
                                        scalar1=sv[:, h:h + 1],
                                        scalar2=None, op0=ALU.mult)
                nc.vector.tensor_add(out=wg[:, h, osl],
                                     in0=wg[:, h, osl], in1=pk)

        # ---- main loop ----
        qts = []
        for h in range(H):
            for j in range(NJ):
                rows = slice(h * 128, (h + 1) * 128)
                cols = slice(j * SEG * D, (j + 1) * SEG * D)
                qt = qpool.tile([128, SEG, D], BF16)
                nc.sync.dma_start(out=qt, in_=q[rows, cols].rearrange(
                    "p (s d) -> p s d", s=SEG))
                qts.append(qt)

        for h in range(H):
            for j in range(NJ):
                t = h * NJ + j
                rows = slice(h * 128, (h + 1) * 128)
                qt = qts[t]
                groups = [(0, 2), (2, 2), (4, 2), (6, 2)]
                for gi, (lo, gn) in enumerate(groups):
                    if (t, gi) in R4_GROUPS:
                        route = "r4"
                    elif (t, gi) in R3_GROUPS:
                        route = "r3"
                    else:
                        route = "r1"

                    if route == "r1":
                        s1h = stat.tile([128, gn], F32, tag=f"as1{gi}")
                        s2h = stat.tile([128, gn], F32, tag=f"as2{gi}")
                        for s in range(lo, lo + gn):
                            i = s - lo
                            nc.vector.scalar_tensor_tensor(
                                out=qt[:, s, :], in0=qt[:, s, :], scalar=1.0,
                                in1=wg[:, h, :], op0=ALU.mult, op1=ALU.add,
                                accum_out=s1h[:, i:i + 1])
                            xsq = work.tile([128, D], BF16, tag="xsq")
                            nc.scalar.activation(
                                out=xsq, in_=qt[:, s, :], func=ACTF.Square,
                                accum_out=s2h[:, i:i + 1])
                        negm = stat.tile([128, gn], F32, tag=f"anm{gi}")
                        nc.gpsimd.tensor_scalar(out=negm, in0=s1h,
                                                scalar1=-1.0 / D,
                                                scalar2=None, op0=ALU.mult)
                        msq = stat.tile([128, gn], F32, tag=f"amq{gi}")
                        nc.gpsimd.tensor_mul(out=msq, in0=negm, in1=negm)
                        var = stat.tile([128, gn], F32, tag=f"avr{gi}")
                        nc.vector.scalar_tensor_tensor(
                            out=var, in0=s2h, scalar=1.0 / D, in1=msq,
                            op0=ALU.mult, op1=ALU.subtract)
                        std = stat.tile([128, gn], F32, tag=f"asd{gi}")
                        nc.scalar.activation(out=std, in_=var, func=ACTF.Sqrt,
                                             bias=eps_t, scale=1.0)
                        rstd = stat.tile([128, gn], F32, tag=f"ars{gi}")
                        nc.vector.reciprocal(out=rstd, in_=std)
                        nmr = stat.tile([128, gn], F32, tag=f"anr{gi}")
                        nc.gpsimd.tensor_mul(out=nmr, in0=negm, in1=rstd)
                        for s in range(lo, lo + gn):
                            i = s - lo
                            sl = slice(i, i + 1)
                            nc.gpsimd.tensor_scalar(
                                out=qt[:, s, :], in0=qt[:, s, :],
                                scalar1=rstd[:, sl], scalar2=nmr[:, sl],
                                op0=ALU.mult, op1=ALU.add)
                            if not ln_trivial:
                                nc.vector.tensor_mul(out=qt[:, s, :],
                                                     in0=qt[:, s, :], in1=lnw_b)
                                nc.vector.tensor_add(out=qt[:, s, :],
                                                     in0=qt[:, s, :], in1=lnb_b)

                    elif route == "r3":
                        mv = stat.tile([128, gn, 2], F32, tag=f"cmv{gi}")
                        for s in range(lo, lo + gn):
                            i = s - lo
                            nc.vector.tensor_add(out=qt[:, s, :],
                                                 in0=qt[:, s, :],
                                                 in1=wg[:, h, :])
                            bnst = work.tile([128, 2, 6], F32, tag="bnst3")
                            nc.vector.bn_stats(out=bnst[:, 0, :],
                                               in_=qt[:, s, 0:320])
                            nc.vector.bn_stats(out=bnst[:, 1, :],
                                               in_=qt[:, s, 320:640])
                            nc.vector.bn_aggr(out=mv[:, i, :], in_=bnst)
                        std = stat.tile([128, gn], F32, tag=f"csd{gi}")
                        nc.scalar.activation(out=std, in_=mv[:, :, 1],
                                             func=ACTF.Sqrt,
                                             bias=eps_t, scale=1.0)
                        rstd = stat.tile([128, gn], F32, tag=f"crs{gi}")
                        nc.vector.reciprocal(out=rstd, in_=std)
                        nmr = stat.tile([128, gn], F32, tag=f"cnr{gi}")
                        nc.vector.scalar_tensor_tensor(
                            out=nmr, in0=mv[:, :, 0], scalar=-1.0,
                            in1=rstd, op0=ALU.mult, op1=ALU.mult)
                        for s in range(lo, lo + gn):
                            i = s - lo
                            sl = slice(i, i + 1)
                            nc.gpsimd.tensor_scalar(
                                out=qt[:, s, :], in0=qt[:, s, :],
                                scalar1=rstd[:, sl], scalar2=nmr[:, sl],
                                op0=ALU.mult, op1=ALU.add)
                            if not ln_trivial:
                                nc.vector.tensor_mul(out=qt[:, s, :],
                                                     in0=qt[:, s, :], in1=lnw_b)
                                nc.vector.tensor_add(out=qt[:, s, :],
                                                     in0=qt[:, s, :], in1=lnb_b)

                    else:
                        mv = stat.tile([128, gn, 2], F32, tag=f"bmv{gi}")
                        pxs = []
                        for s in range(lo, lo + gn):
                            i = s - lo
                            px = psum_x.tile([128, D], F32, tag="x")
                            # matmul PSUM dest must stay within one bank:
                            # split 640 into 512 + 128 bank-aligned chunks
                            for c0, c1 in ((0, 512), (512, 640)):
                                nc.tensor.matmul(px[:, c0:c1], lhsT=id_sb,
                                                 rhs=qt[:, s, c0:c1],
                                                 start=True, stop=False)
                                nc.tensor.matmul(px[:, c0:c1], lhsT=id_sb,
                                                 rhs=wg[:, h, c0:c1],
                                                 start=False, stop=True)
                            bnst = work.tile([128, 2, 6], F32, tag="bnst")
                            nc.vector.bn_stats(out=bnst[:, 0, :],
                                               in_=px[:, 0:512])
                            nc.vector.bn_stats(out=bnst[:, 1, :],
                                               in_=px[:, 512:640])
                            nc.vector.bn_aggr(out=mv[:, i, :], in_=bnst)
                            pxs.append(px)
                        std = stat.tile([128, gn], F32, tag=f"bsd{gi}")
                        nc.scalar.activation(out=std, in_=mv[:, :, 1],
                                             func=ACTF.Sqrt,
                                             bias=eps_t, scale=1.0)
                        rstd = stat.tile([128, gn], F32, tag=f"brs{gi}")
                        nc.vector.reciprocal(out=rstd, in_=std)
                        nmr = stat.tile([128, gn], F32, tag=f"bnr{gi}")
                        nc.vector.scalar_tensor_tensor(
                            out=nmr, in0=mv[:, :, 0], scalar=-1.0,
                            in1=rstd, op0=ALU.mult, op1=ALU.mult)
                        for s in range(lo, lo + gn):
                            i = s - lo
                            sl = slice(i, i + 1)
                            nc.scalar.activation(
                                out=qt[:, s, :], in_=pxs[i],
                                func=ACTF.Identity,
                                bias=nmr[:, sl], scale=rstd[:, sl])
                            if not ln_trivial:
                                nc.vector.tensor_mul(out=qt[:, s, :],
                                                     in0=qt[:, s, :], in1=lnw_b)
                                nc.vector.tensor_add(out=qt[:, s, :],
                                                     in0=qt[:, s, :], in1=lnb_b)

                    ch = slice(j * SEG * D + lo * D,
                               j * SEG * D + (lo + gn) * D)
                    nc.sync.dma_start(out=o[rows, ch].rearrange(
                        "p (s d) -> p s d", s=gn), in_=qt[:, lo:lo + gn, :])

    nc.finalize()
    return nc


_NC_CACHE: dict = {}


def _prepare(q, k, v, w_fc, b_fc, gamma1, ln_w, ln_b):
    qf = np.asarray(q, np.float32).reshape(NUM_C, LQ * D) \
        .astype(ml_dtypes.bfloat16)
    kf = np.ascontiguousarray(np.asarray(k, np.float32)).reshape(NUM_C, D)
    vf = np.ascontiguousarray(np.asarray(v, np.float32)).reshape(NUM_C, D)
    g = np.asarray(gamma1, np.float32)
    wgw_full = np.asarray(w_fc, np.float32).T * g[None, :]   # (D_in, D_out)
    wgw = np.ascontiguousarray(
        wgw_full.reshape(5, 128, D).transpose(1, 0, 2)
        .astype(ml_dtypes.bfloat16))
    wgb = np.ascontiguousarray(
        (np.asarray(b_fc, np.float32) * g).reshape(1, D)
        .astype(ml_dtypes.bfloat16))
    ident = np.eye(128, dtype=ml_dtypes.bfloat16)
    lnw = np.asarray(ln_w, np.float32)
    lnb = np.asarray(ln_b, np.float32)
    ln_trivial = bool(np.all(lnw == 1.0) and np.all(lnb == 0.0))

    in_maps = []
    for i in range(N_CORES):
        rows = slice(i * B, (i + 1) * B)
        vT = np.ascontiguousarray(
            vf[rows].T.reshape(5, 128, B).transpose(1, 0, 2)
            .astype(ml_dtypes.bfloat16))
        kT = np.ascontiguousarray(
            kf[rows].T.reshape(5, 128, B).transpose(1, 0, 2)
            .astype(ml_dtypes.bfloat16))
        m = {"q": np.ascontiguousarray(qf[rows]),
             "vv": vf[rows].astype(ml_dtypes.bfloat16),
             "vT": vT, "kT": kT, "wgw": wgw, "wgb": wgb, "ident": ident}
        if not ln_trivial:
            m["lnw"] = lnw.reshape(1, D)
            m["lnb"] = lnb.reshape(1, D)
        in_maps.append(m)
    return in_maps, ln_trivial


def _postprocess(results):
    return np.concatenate(
        [r["o"].astype(np.float32).reshape(B, LQ, D) for r in results],
        axis=0)


def run(inputs: dict, trace: bool = False, tmpdir=None):
    in_maps, ln_trivial = _prepare(**inputs)
    key = ln_trivial
    if key not in _NC_CACHE:
        _NC_CACHE[key] = _build(ln_trivial)
    nc = _NC_CACHE[key]
    res = run_bass_kernel_spmd(nc, in_maps, core_ids=list(range(N_CORES)),
                               trace=trace, tmpdir=tmpdir)
    return _postprocess(res.results), res


def kernel(**inputs) -> np.ndarray:
    out, _ = run(inputs, trace=False)
    return out

